# revision 19
# baseline (speedup 1.0000x reference)
"""DiffAttn kernel for 8 Trainium2 NeuronCores.

Sharding: core c -> (batch b = c//2, query-half h = c%2). Each core computes
2048 query rows of both score matrices against the full K/V of its batch.
K/V projections are computed once per batch: each core projects only its
own 2048 rows and the halves are exchanged with the pair peer via a DRAM
AllGather (replica groups [[0,1],[2,3],[4,5],[6,7]]); the gathered member
order equals the global row order, so the SPMD program stays symmetric.

Per-core pipeline (all matmul inputs fp16, fp32 PSUM accumulate):
  1. DMA-transpose (16-bit xbar) X and weights into contraction-major layout.
  2. PE projections -> QT/KT [feature, seq] fp16, V [seq, d] fp16.
  3. Scores computed transposed: A.T[k, q] = KT_tile.T @ QT_tile, ACT exp
     writes E.T tiles for BOTH matrices to SBUF; row sums ride on DVE
     (acc += E.T tile) with a gpsimd cross-partition all-reduce giving the
     row-sum in broadcast orientation.  The two softmaxes are then combined
     BEFORE the P@V matmul:  D = E1 - (lam*s1/s2)[q] * E2, so only ONE
     P@V pass is needed; the 1/s1 normalization is applied per-partition
     after P@V.  This removes 1/4 of all attention matmul work.
"""

import math
import os

import numpy as np

import concourse.bacc as bacc
import concourse.mybir as mybir
import concourse.tile as tile
from concourse import bass_isa
from concourse.bass_utils import run_bass_kernel_spmd

F32 = mybir.dt.float32
F16 = mybir.dt.float16
AF = mybir.ActivationFunctionType
ALU = mybir.AluOpType

B, S, E, D = 4, 4096, 1024, 512
TWO_D = 2 * D
QR = S // 2          # query rows per core
QB = 512             # query block in attention
P = 128
N_E = E // P         # 8 contraction chunks over E
N_F = TWO_D // P     # 8 feature chunks for Q/K
KC = S // P          # 32 key chunks
NQS = QB // P        # 4 q sub-blocks per query block
LAMBDA_INIT = 0.05
S_SCALE = 1.0 / math.sqrt(D)

LAST_RESULTS = None


def _emit(nc, tc, ctx, reps=1, bench=False):
    # bench=True: declare I/O as Internal DRAM (garbage contents) so timing
    # runs skip the large axon input upload / output download.
    kin = dict(kind="Internal") if bench else dict(kind="ExternalInput")
    kout = dict(kind="Internal") if bench else dict(kind="ExternalOutput")
    Xf = nc.dram_tensor("Xf", [QR, E], F32, **kin).ap()
    Wq = nc.dram_tensor("Wq", [TWO_D, E], F32, **kin).ap()
    Wk = nc.dram_tensor("Wk", [TWO_D, E], F32, **kin).ap()
    Wv = nc.dram_tensor("Wv", [D, E], F32, **kin).ap()
    bq = nc.dram_tensor("bq", [TWO_D, 1], F32, **kin).ap()
    bk = nc.dram_tensor("bk", [TWO_D, 1], F32, **kin).ap()
    bv = nc.dram_tensor("bv", [1, D], F32, **kin).ap()
    lam = nc.dram_tensor("lam", [1, 1], F32, **kin).ap()
    out = nc.dram_tensor("out", [QR, D], F32, **kout).ap()
    if bench:
        dummy = nc.dram_tensor("bench_out", [1, 4], F32, kind="ExternalOutput").ap()

    const = ctx.enter_context(tc.tile_pool(name="const", bufs=1))
    resident = ctx.enter_context(tc.tile_pool(name="resident", bufs=1))
    dram = ctx.enter_context(tc.tile_pool(name="dram", bufs=1, space="DRAM"))
    ps_work = ctx.enter_context(tc.tile_pool(name="ps_work", bufs=3, space="PSUM"))
    ps_out = ctx.enter_context(tc.tile_pool(name="ps_out", bufs=4, space="PSUM"))
    ps_sums = ctx.enter_context(tc.tile_pool(name="ps_sums", bufs=1, space="PSUM"))

    for _rep in range(reps):
        # ---- constants / small inputs ----
        bqc = const.tile([P, N_F], F32, tag="bqc")
        bkc = const.tile([P, N_F], F32, tag="bkc")
        for c in range(N_F):
            nc.sync.dma_start(bqc[:, c : c + 1], bq[c * P : (c + 1) * P, :])
            nc.sync.dma_start(bkc[:, c : c + 1], bk[c * P : (c + 1) * P, :])
        bv32 = const.tile([1, D], F32, tag="bv32")
        nc.sync.dma_start(bv32[:], bv[:])
        bv16 = const.tile([1, D], F16, tag="bv16")
        nc.vector.tensor_copy(bv16[:], bv32[:])

        lam32 = const.tile([1, 1], F32, tag="lam32")
        nc.sync.dma_start(lam32[:], lam[:])
        lam_e = const.tile([1, 1], F32, tag="lam_e")
        nc.scalar.activation(lam_e[:], lam32[:], AF.Exp)
        lam_p = const.tile([1, 1], F32, tag="lam_p")
        # lam_p = exp(lam) + LAMBDA_INIT
        nc.vector.tensor_scalar(lam_p[:], lam_e[:], LAMBDA_INIT, None, ALU.add)
        plam = const.tile([P, 1], F32, tag="plam")
        nc.gpsimd.partition_broadcast(plam[:], lam_p[:])

        ones_col = const.tile([P, 1], F16, tag="ones_col")
        nc.vector.memset(ones_col[:], 1.0)
        bvb = const.tile([P, D], F16, tag="bvb")
        nc.gpsimd.partition_broadcast(bvb[:], bv16[:])

        # ---- resident tensors ----
        kt = resident.tile([P, N_F, S], F16, tag="kt", name="kt")       # [f, k]
        qt = resident.tile([P, N_F, QR], F16, tag="qt", name="qt")      # [f, q]
        vt = resident.tile([P, KC, D], F16, tag="vt", name="vt")        # [k, d]

        # fp16 staging in DRAM (HWDGE load -> DVE cast -> HWDGE write),
        # then large xbar transposes.
        xf16 = dram.tile([QR, E], F16, tag="xf16", name=f"xf16_{_rep}")
        kvsend_k = dram.tile([P, N_F * QR], F16, tag="kvsk", name=f"kvsk_{_rep}")
        kvrecv_k = dram.tile([2 * P, N_F * QR], F16, tag="kvrk", name=f"kvrk_{_rep}")
        kvsend_v = dram.tile([P, (KC // 2) * D], F16, tag="kvsv", name=f"kvsv_{_rep}")
        kvrecv_v = dram.tile([2 * P, (KC // 2) * D], F16, tag="kvrv", name=f"kvrv_{_rep}")
        w16q = dram.tile([TWO_D, E], F16, tag="w16q", name=f"w16q_{_rep}")
        w16k = dram.tile([TWO_D, E], F16, tag="w16k", name=f"w16k_{_rep}")
        w16v = dram.tile([D, E], F16, tag="w16v", name=f"w16v_{_rep}")

        # ---------------- projection phase ----------------
        import contextlib

        with contextlib.ExitStack() as pctx:
            wres = pctx.enter_context(tc.tile_pool(name=f"wres{_rep}", bufs=1))
            xstage = pctx.enter_context(tc.tile_pool(name=f"xstage{_rep}", bufs=3))
            xcast = pctx.enter_context(tc.tile_pool(name=f"xcast{_rep}", bufs=2))
            xtp = pctx.enter_context(tc.tile_pool(name=f"xtp{_rep}", bufs=2))

            wqt = wres.tile([P, N_E, TWO_D], F16, tag="wqt")   # [e, f]
            wkt = wres.tile([P, N_E, TWO_D], F16, tag="wkt")
            wvt = wres.tile([P, N_E, D], F16, tag="wvt")

            def cast_to_dram(src_dram, dst16, rows, r0=0):
                for rc in range(r0, rows // P):
                    st = xstage.tile([P, E], F32, tag="xs")
                    nc.scalar.dma_start(st[:], src_dram[rc * P : (rc + 1) * P, :])
                    cst = xcast.tile([P, E], F16, tag="xc")
                    nc.vector.tensor_copy(cst[:], st[:])
                    nc.scalar.dma_start(dst16[rc * P : (rc + 1) * P, :], cst[:])

            def w_transpose(w16_, w_t, rows):
                # split per row chunk: transpose rc becomes ready as soon as
                # cast chunk rc lands, so the first projection matmul does
                # not wait for the whole weight to stage
                for rc in range(rows // P):
                    for eo in range(N_E):
                        nc.sync.dma_start(
                            w_t[:, eo, rc * P : (rc + 1) * P],
                            w16_[rc * P : (rc + 1) * P, eo * P : (eo + 1) * P],
                            transpose=True,
                        )

            # Wk chunk 0 + first X block first: K-proj fo=0 depends only on
            # these, so PE starts ~10us earlier.
            cast_to_dram(Wk, w16k, P)
            cast_to_dram(Xf, xf16, 512)  # first X block
            cast_to_dram(Wk, w16k, TWO_D, r0=1)
            cast_to_dram(Wv, w16v, D)
            w_transpose(w16k, wkt, TWO_D)
            w_transpose(w16v, wvt, D)

            def load_xt_block(sb):
                """Transpose 512 fp16 X rows (sb*512..) into an [e, s] block."""
                xt_blk = xtp.tile([P, N_E, 512], F16, tag="xt")
                for eo in range(N_E):
                    nc.sync.dma_start(
                        xt_blk[:, eo, :],
                        xf16[sb * 512 : (sb + 1) * 512, eo * P : (eo + 1) * P],
                        transpose=True,
                    )
                return xt_blk

            # ---- K / V projections over OWN rows only; pair-exchange the
            # halves via AllGather so each batch's K/V is computed once ----
            kvst = pctx.enter_context(tc.tile_pool(name=f"kvst{_rep}", bufs=3))
            for sb in range(QR // 512):
                if 1 <= sb < QR // 512 - 1:
                    # stage the *next* X block while computing on this one
                    for rc in range(4):
                        row = (sb + 1) * 512 + rc * P
                        st = xstage.tile([P, E], F32, tag="xs")
                        nc.scalar.dma_start(st[:], Xf[row : row + P, :])
                        cst = xcast.tile([P, E], F16, tag="xc")
                        nc.vector.tensor_copy(cst[:], st[:])
                        nc.scalar.dma_start(xf16[row : row + P, :], cst[:])
                elif sb == 0:
                    for rc in range(4, 8):
                        row = rc * P
                        st = xstage.tile([P, E], F32, tag="xs")
                        nc.scalar.dma_start(st[:], Xf[row : row + P, :])
                        cst = xcast.tile([P, E], F16, tag="xc")
                        nc.vector.tensor_copy(cst[:], st[:])
                        nc.scalar.dma_start(xf16[row : row + P, :], cst[:])

                xt_blk = load_xt_block(sb)
                for fo in range(N_F):
                    ps = ps_work.tile([P, 512], F32, tag="work")
                    for eo in range(N_E):
                        nc.tensor.matmul(
                            ps[:],
                            wkt[:, eo, fo * P : (fo + 1) * P],
                            xt_blk[:, eo, :],
                            start=eo == 0,
                            stop=eo == N_E - 1,
                        )
                    kst = kvst.tile([P, 512], F16, tag="kv")
                    nc.scalar.activation(
                        kst[:], ps[:], AF.Identity, bias=bkc[:, fo : fo + 1]
                    )
                    nc.scalar.dma_start(
                        kvsend_k[:, fo * QR + sb * 512 : fo * QR + (sb + 1) * 512],
                        kst[:],
                    )
                for ssub in range(4):
                    ps = ps_work.tile([P, 512], F32, tag="work")
                    for eo in range(N_E):
                        nc.tensor.matmul(
                            ps[:],
                            xt_blk[:, eo, ssub * P : (ssub + 1) * P],
                            wvt[:, eo, :],
                            start=eo == 0,
                            stop=eo == N_E - 1,
                        )
                    vst = kvst.tile([P, 512], F16, tag="kv")
                    nc.vector.tensor_tensor(vst[:], ps[:], bvb[:], ALU.add)
                    kc_l = sb * 4 + ssub
                    nc.scalar.dma_start(
                        kvsend_v[:, kc_l * D : (kc_l + 1) * D], vst[:]
                    )

            # pair exchange: member order == global row order on both cores
            groups = [[0, 1], [2, 3], [4, 5], [6, 7]]
            nc.gpsimd.collective_compute(
                "AllGather", ALU.bypass, replica_groups=groups,
                ins=[kvsend_k[:]], outs=[kvrecv_k[:]],
            )
            nc.gpsimd.collective_compute(
                "AllGather", ALU.bypass, replica_groups=groups,
                ins=[kvsend_v[:]], outs=[kvrecv_v[:]],
            )
            for mem in range(2):
                nc.sync.dma_start(
                    kt[:, :, mem * QR : (mem + 1) * QR],
                    kvrecv_k[mem * P : (mem + 1) * P, :],
                )
                nc.sync.dma_start(
                    vt[:, mem * (KC // 2) : (mem + 1) * (KC // 2), :],
                    kvrecv_v[mem * P : (mem + 1) * P, :],
                )

            # ---- Q projection over this core's 2048 rows (reuses xf16) ----
            cast_to_dram(Wq, w16q, TWO_D)
            w_transpose(w16q, wqt, TWO_D)
            # own query rows are xf16 rows 0..QR-1 (inputs are pre-rotated)
            for sb in range(QR // 512):
                xt_blk = load_xt_block(sb)
                for fo in range(N_F):
                    ps = ps_work.tile([P, 512], F32, tag="work")
                    for eo in range(N_E):
                        nc.tensor.matmul(
                            ps[:],
                            wqt[:, eo, fo * P : (fo + 1) * P],
                            xt_blk[:, eo, :],
                            start=eo == 0,
                            stop=eo == N_E - 1,
                        )
                    nc.scalar.activation(
                        qt[:, fo, sb * 512 : (sb + 1) * 512],
                        ps[:],
                        AF.Identity,
                        bias=bqc[:, fo : fo + 1],
                    )

        # ---------------- attention phase ----------------
        with contextlib.ExitStack() as actx:
            e1p = actx.enter_context(tc.tile_pool(name=f"e1p{_rep}", bufs=KC))
            e2p = actx.enter_context(tc.tile_pool(name=f"e2p{_rep}", bufs=KC))
            accp = actx.enter_context(tc.tile_pool(name=f"accp{_rep}", bufs=2))
            srp = actx.enter_context(tc.tile_pool(name=f"srp{_rep}", bufs=2))
            cbp = actx.enter_context(tc.tile_pool(name=f"cbp{_rep}", bufs=1))
            dtp = actx.enter_context(tc.tile_pool(name=f"dtp{_rep}", bufs=3))
            tmpp = actx.enter_context(tc.tile_pool(name=f"tmpp{_rep}", bufs=1))
            finp = actx.enter_context(tc.tile_pool(name=f"finp{_rep}", bufs=1))
            rp = actx.enter_context(tc.tile_pool(name=f"rp{_rep}", bufs=2))

            for qb in range(QR // QB):
                accs = []
                e_tiles = {}
                s1b = None
                for m in range(2):
                    acc = accp.tile([P, QB], F16, tag="acc")
                    accs.append(acc)
                    ep = e1p if m == 0 else e2p
                    for kc in range(KC):
                        a_ps = ps_work.tile([P, QB], F32, tag="work")
                        for dd in range(4):
                            fo = m * 4 + dd
                            nc.tensor.matmul(
                                a_ps[:],
                                kt[:, fo, kc * P : (kc + 1) * P],
                                qt[:, fo, qb * QB : (qb + 1) * QB],
                                start=dd == 0,
                                stop=dd == 3,
                            )
                        et = ep.tile([P, QB], F16, tag="e")
                        nc.scalar.activation(et[:], a_ps[:], AF.Exp, scale=S_SCALE)
                        e_tiles[(m, kc)] = et
                        if kc == 0:
                            nc.vector.tensor_copy(acc[:], et[:])
                        else:
                            nc.vector.tensor_tensor(acc[:], acc[:], et[:], ALU.add)
                    if m == 0:
                        # matrix-1 sums overlap matrix-2's score matmuls:
                        # only s2's short chain remains on the qb boundary
                        s1b = srp.tile([P, QB], F16, tag="sr")
                        nc.gpsimd.partition_all_reduce(
                            s1b[:], accs[0][:], 128, bass_isa.ReduceOp.add
                        )
                        sums_col = ps_sums.tile([P, NQS], F32, tag="sc")
                        for qs in range(NQS):
                            nc.tensor.matmul(
                                sums_col[:, qs : qs + 1],
                                accs[0][:, qs * P : (qs + 1) * P],
                                ones_col[:],
                                start=qs == 0,
                                stop=qs == NQS - 1,
                            )
                        r1c = rp.tile([P, NQS], F32, tag="r")
                        nc.vector.reciprocal(r1c[:], sums_col[:])

                s2b = srp.tile([P, QB], F16, tag="sr")
                nc.gpsimd.partition_all_reduce(
                    s2b[:], accs[1][:], 128, bass_isa.ReduceOp.add
                )

                # Cb[p, q] = lam * s1[q] / s2[q]  (reciprocal computed in-place)
                with nc.allow_low_precision(reason="f16 recip of row sums, 5e-4 rel"):
                    nc.vector.reciprocal(s2b[:], s2b[:])
                cb = cbp.tile([P, QB], F16, tag="cb")
                nc.vector.scalar_tensor_tensor(
                    cb[:], s2b[:], plam[:, 0:1], s1b[:], ALU.mult, ALU.mult
                )

                outp = [
                    ps_out.tile([P, D], F32, tag="out", name=f"out{qb}_{qs}")
                    for qs in range(NQS)
                ]
                for kc in range(KC):
                    t = tmpp.tile([P, QB], F16, tag="t")
                    nc.vector.tensor_tensor(t[:], e_tiles[(1, kc)][:], cb[:], ALU.mult)
                    dt = dtp.tile([P, QB], F16, tag="d")
                    nc.vector.tensor_tensor(dt[:], e_tiles[(0, kc)][:], t[:], ALU.subtract)
                    for qs in range(NQS):
                        nc.tensor.matmul(
                            outp[qs][:],
                            dt[:, qs * P : (qs + 1) * P],
                            vt[:, kc, :],
                            start=kc == 0,
                            stop=kc == KC - 1,
                        )

                for qs in range(NQS):
                    fin = finp.tile([P, D], F32, tag="fin")
                    nc.vector.tensor_scalar(
                        fin[:], outp[qs][:], r1c[:, qs : qs + 1], None, ALU.mult
                    )
                    row0 = qb * QB + qs * P
                    nc.sync.dma_start(out[row0 : row0 + P, :], fin[:])

        if bench:
            nc.sync.dma_start(dummy[:], bv32[0:1, 0:4])


_NC_CACHE = {}


def _get_nc(reps=1, bench=False):
    key = (reps, bench)
    if key not in _NC_CACHE:
        nc = bacc.Bacc("TRN2", target_bir_lowering=False, debug=False, num_devices=8)
        with tile.TileContext(nc) as tc:
            with __import__("contextlib").ExitStack() as ctx:
                _emit(nc, tc, ctx, reps=reps, bench=bench)
        nc.compile()
        _NC_CACHE[key] = nc
    return _NC_CACHE[key]


def build_in_maps(X, Wq, bq, Wk, bk, Wv, bv, lam, **_unused):
    X = np.asarray(X, dtype=np.float32)
    Wq = np.ascontiguousarray(np.asarray(Wq, dtype=np.float32))
    Wk = np.ascontiguousarray(np.asarray(Wk, dtype=np.float32))
    Wv = np.ascontiguousarray(np.asarray(Wv, dtype=np.float32))
    bq_ = np.ascontiguousarray(np.asarray(bq, dtype=np.float32).reshape(TWO_D, 1))
    bk_ = np.ascontiguousarray(np.asarray(bk, dtype=np.float32).reshape(TWO_D, 1))
    bv_ = np.ascontiguousarray(np.asarray(bv, dtype=np.float32).reshape(1, D))
    lam_ = np.ascontiguousarray(np.asarray(lam, dtype=np.float32).reshape(1, 1))

    in_maps = []
    for c in range(8):
        b, h = c // 2, c % 2
        # Each core receives ONLY its own 2048 rows; the peer's K/V half
        # arrives via the pair AllGather (member order == global row order).
        in_maps.append(
            {
                "Xf": np.ascontiguousarray(X[b, h * QR : (h + 1) * QR]),
                "Wq": Wq,
                "Wk": Wk,
                "Wv": Wv,
                "bq": bq_,
                "bk": bk_,
                "bv": bv_,
                "lam": lam_,
            }
        )
    return in_maps


def kernel(X, Wq, bq, Wk, bk, Wv, bv, lam, **_unused):
    global LAST_RESULTS
    nc = _get_nc()
    in_maps = build_in_maps(X, Wq, bq, Wk, bk, Wv, bv, lam)
    trace = bool(int(os.environ.get("DIFFATTN_TRACE", "0")))
    res = run_bass_kernel_spmd(nc, in_maps, core_ids=list(range(8)), trace=trace)
    LAST_RESULTS = res
    full = np.empty((B, S, D), dtype=np.float32)
    for c in range(8):
        b, h = c // 2, c % 2
        full[b, h * QR : (h + 1) * QR] = res.results[c]["out"]
    return full
